# revision 1
# baseline (speedup 1.0000x reference)
"""GNN bi-interaction aggregator (gnn_message_passing) on 8 trn2 NeuronCores.

reference:
    msgs = edge_val[:, None] * embeddings[edge_col]          # [E, D]
    side = segment_sum(msgs, edge_row, N)                    # [N, D]
    out  = lrelu((emb + side) @ W_sum.T + b_sum)
         + lrelu((emb * side) @ W_prod.T + b_prod)

Sharding: row-partition destinations across 8 cores (6250 nodes each); each
core receives the full embedding table (bf16) in its DRAM plus its own edge
shard, so no collectives are needed.

Per-core algorithm:
  - host sorts the core's edges by destination into 128-segment windows and
    into two source classes (src < 32768 / >= 32768, the int16 dma_gather
    limit), padded to 128-edge tiles with a cross-core-uniform tile schedule
    (SPMD: one program for all cores; counts are max over cores).
  - chunked dma_gather pulls source rows (bf16, 256B/row) into SBUF.
  - per tile, one fused DVE tensor_scalar builds S[e, s] = val[e] *
    (iota[s] == destrel[e]); PE accumulates sideT[d, s] += msgs.T @ S in a
    PSUM bank per window (start/stop accumulation groups).
  - ACT drains each window to sideT (bf16); emb is PE-transposed to embT;
    X1T = sideT + embT, X2T = embT * sideT on DVE; node-major downstream
    matmuls (lhsT = X1T/X2T blocks, rhs = W.T) + Lrelu on ACT.
"""
import math
import numpy as np
import ml_dtypes

import concourse.bass as bass
import concourse.bacc as bacc
import concourse.mybir as mybir
import concourse.tile as tile
from concourse.bass_utils import run_bass_kernel_spmd
from concourse.masks import make_identity

N_NODES = 50000
N_EDGES = 600000
D = 128
NCORES = 8
NPC = N_NODES // NCORES          # 6250 destinations per core
WSEG = 128                       # segments per PSUM window
NW = math.ceil(NPC / WSEG)       # 49 windows per core
SPLIT = 32768                    # source class boundary (int16 idx limit)
CHUNK_W = 4                      # windows per gather chunk
NCHUNK = math.ceil(NW / CHUNK_W)
SLOPE = 0.01
BF16 = ml_dtypes.bfloat16
SIM_RELU = False   # CoreSim lacks Lrelu; set True only in sim tests


def _wrap_idx(idx: np.ndarray) -> np.ndarray:
    """dma_gather idx layout: arr[p, s] = idx[s*16 + p%16], [128, n/16] int16."""
    n = len(idx)
    assert n % 16 == 0
    a16 = idx.astype(np.int16).reshape(-1, 16).T          # [16, n/16]
    return np.ascontiguousarray(np.tile(a16, (8, 1)))     # [128, n/16]


def _preprocess(edge_row, edge_col, edge_val):
    """Index-only preprocessing -> per-core arrays + uniform tile schedule."""
    core = edge_row // NPC
    er = edge_row - core * NPC           # dest local to core
    w = er // WSEG                       # window id
    destrel = er - w * WSEG              # seg within window, [0, 128)
    cls = (edge_col >= SPLIT).astype(np.int64)   # 0=A, 1=B

    # counts[core, cls, w]
    key = (core * 2 + cls) * NW + w
    counts = np.bincount(key, minlength=NCORES * 2 * NW).reshape(NCORES, 2, NW)
    tiles = -(-counts // 128)            # ceil div
    T = tiles.max(axis=0)                # [2, NW] uniform tile counts
    T = np.maximum(T, 1)                 # >=1 tile per (class, window)

    n_slots = T * 128                    # [2, NW]
    slot_off = np.zeros((2, NW), dtype=np.int64)
    slot_off[0, 1:] = np.cumsum(n_slots[0])[:-1]
    slot_off[1, 1:] = np.cumsum(n_slots[1])[:-1]
    tot_slots = n_slots.sum(axis=1)      # [2]
    TA, TB = int(T[0].sum()), int(T[1].sum())

    per_core = []
    for c in range(NCORES):
        m = core == c
        ccls, cw = cls[m], w[m]
        csrc, cdr, cval = edge_col[m], destrel[m], edge_val[m]
        order = np.lexsort((cw, ccls))
        ccls, cw, csrc, cdr, cval = (
            ccls[order], cw[order], csrc[order], cdr[order], cval[order])

        slots_idx = [np.zeros(tot_slots[0], np.int64),
                     np.zeros(tot_slots[1], np.int64)]
        slots_dr = [np.zeros(tot_slots[0], np.float32),
                    np.zeros(tot_slots[1], np.float32)]
        slots_val = [np.zeros(tot_slots[0], np.float32),
                     np.zeros(tot_slots[1], np.float32)]
        # group boundaries: edges sorted by (cls, w)
        grp = np.searchsorted(ccls * NW + cw, np.arange(2 * NW + 1))
        for x in range(2):
            for wi in range(NW):
                lo, hi = grp[x * NW + wi], grp[x * NW + wi + 1]
                n = hi - lo
                if n == 0:
                    continue
                o = slot_off[x, wi]
                slots_idx[x][o:o + n] = csrc[lo:hi] - (SPLIT if x else 0)
                slots_dr[x][o:o + n] = cdr[lo:hi]
                slots_val[x][o:o + n] = cval[lo:hi]

        # host-precomputed S: S[slot, seg] = val if destrel == seg else 0
        s_parts = []
        for x, Tn in ((0, TA), (1, TB)):
            ns = Tn * 128
            s = np.zeros((ns, WSEG), dtype=np.float32)
            rows = np.nonzero(slots_val[x])[0]
            s[rows, slots_dr[x][rows].astype(np.int64)] = slots_val[x][rows]
            # [Tn tiles, 128 slots, WSEG] -> [128 slots, Tn*WSEG]
            s_parts.append(np.ascontiguousarray(
                s.reshape(Tn, 128, WSEG).transpose(1, 0, 2).reshape(128, Tn * WSEG)))
        S_cols = np.concatenate(s_parts, axis=1).astype(BF16)
        per_core.append(dict(
            idxA=_wrap_idx(slots_idx[0]),
            idxB=_wrap_idx(slots_idx[1]),
            S=np.ascontiguousarray(S_cols),
        ))
    sched = dict(T=T, slot_off=slot_off, tot_slots=tot_slots, TA=TA, TB=TB)
    return per_core, sched


def _act_kw():
    if SIM_RELU:
        return dict(func=mybir.ActivationFunctionType.Relu)
    return dict(func=mybir.ActivationFunctionType.Lrelu, alpha=SLOPE)


def _build(nc, sched, with_bias):
    T, slot_off = sched["T"], sched["slot_off"]
    tot_slots, TA, TB = sched["tot_slots"], sched["TA"], sched["TB"]
    Ttot = TA + TB
    bf = mybir.dt.bfloat16
    f32 = mybir.dt.float32

    t_table = nc.declare_dram_parameter("table", [N_NODES, D], bf, isOutput=False)
    t_emb = nc.declare_dram_parameter("emb_own", [NPC, D], bf, isOutput=False)
    t_idxA = nc.declare_dram_parameter("idxA", [128, tot_slots[0] // 16],
                                       mybir.dt.int16, isOutput=False)
    t_idxB = nc.declare_dram_parameter("idxB", [128, tot_slots[1] // 16],
                                       mybir.dt.int16, isOutput=False)
    t_S = nc.declare_dram_parameter("S", [128, Ttot * WSEG], bf, isOutput=False)
    t_wsum = nc.declare_dram_parameter("wsumT", [D, D], bf, isOutput=False)
    t_wprod = nc.declare_dram_parameter("wprodT", [D, D], bf, isOutput=False)
    if with_bias:
        t_bias = nc.declare_dram_parameter("biases", [128, 2 * D], f32,
                                           isOutput=False)
    t_out = nc.declare_dram_parameter("out", [NPC, D], f32, isOutput=True)

    NCOL = NW * WSEG  # 6272 columns in T-space buffers

    # chunk boundaries (tiles and slots), per class
    chunk_tile_lo = np.zeros((2, NCHUNK + 1), dtype=np.int64)
    for x in range(2):
        ct = np.cumsum(np.concatenate([[0], T[x]]))
        for k in range(NCHUNK + 1):
            chunk_tile_lo[x, k] = ct[min(k * CHUNK_W, NW)]

    with tile.TileContext(nc) as tc:
        with (
            tc.tile_pool(name="cst", bufs=1) as cst,
            tc.tile_pool(name="gbufA", bufs=6) as gbufA,
            tc.tile_pool(name="gbufB", bufs=6) as gbufB,
            tc.tile_pool(name="sSA", bufs=2) as sSA,
            tc.tile_pool(name="sSB", bufs=2) as sSB,
            tc.tile_pool(name="small", bufs=3) as small,
            tc.tile_pool(name="psw", bufs=3, space="PSUM") as psw,
            tc.tile_pool(name="psd", bufs=3, space="PSUM") as psd,
            tc.tile_pool(name="pst", bufs=2, space="PSUM") as pst,
        ):
            # ---- statics; chunk-0 idx columns first so gathers start ASAP
            c0A = int(chunk_tile_lo[0, 1]) * 8
            c0B = int(chunk_tile_lo[1, 1]) * 8
            idxA = cst.tile([128, tot_slots[0] // 16], mybir.dt.int16)
            nc.sync.dma_start(out=idxA[:, :c0A], in_=t_idxA[:, :c0A])
            idxB = cst.tile([128, tot_slots[1] // 16], mybir.dt.int16)
            nc.sync.dma_start(out=idxB[:, :c0B], in_=t_idxB[:, :c0B])
            nc.sync.dma_start(out=idxA[:, c0A:], in_=t_idxA[:, c0A:])
            nc.sync.dma_start(out=idxB[:, c0B:], in_=t_idxB[:, c0B:])
            wsum = cst.tile([D, D], bf)
            nc.sync.dma_start(out=wsum[:], in_=t_wsum[:])
            wprod = cst.tile([D, D], bf)
            nc.sync.dma_start(out=wprod[:], in_=t_wprod[:])
            if with_bias:
                bias = cst.tile([128, 2 * D], f32)
                nc.sync.dma_start(out=bias[:], in_=t_bias[:])
            ident = cst.tile([128, 128], bf)
            make_identity(nc, ident[:])
            sideT = cst.tile([D, NCOL], bf)
            embT = cst.tile([D, NCOL], bf)
            x2T = cst.tile([D, NCOL], bf)

            srcA = t_table[:SPLIT, :]
            srcB = t_table[SPLIT:, :]
            tile_base = [np.cumsum(np.concatenate([[0], T[x]])) for x in range(2)]

            # ---- embT: PE-transpose own embeddings (overlaps first gathers)
            NB = math.ceil(NPC / 128)  # 49 blocks, last partial (106 rows)
            for b in range(NB):
                r0 = b * 128
                nrow = min(128, NPC - r0)
                eb = small.tile([128, D], bf, tag="eb")
                if nrow < 128:
                    nc.vector.memset(eb[:], 0)
                nc.sync.dma_start(out=eb[:nrow, :], in_=t_emb[r0 : r0 + nrow, :])
                pt = pst.tile([D, 128], bf, space="PSUM", tag="pt")
                nc.tensor.transpose(pt[:], eb[:], ident[:])
                nc.scalar.activation(embT[:, r0 : r0 + 128], pt[:],
                                     mybir.ActivationFunctionType.Copy)

            # ---- scatter, chunk-pipelined; X fused from PSUM per window;
            # downstream for chunk k-1 emitted after chunk k's gathers
            def downstream_block(b):
                r0 = b * 128
                nrow = min(128, NPC - r0)
                p1 = psd.tile([128, D], f32, space="PSUM", tag="pd")
                nc.tensor.matmul(p1[:nrow, :], lhsT=sideT[:, r0 : r0 + nrow],
                                 rhs=wsum[:], start=True, stop=True)
                p2 = psd.tile([128, D], f32, space="PSUM", tag="pd")
                nc.tensor.matmul(p2[:nrow, :], lhsT=x2T[:, r0 : r0 + nrow],
                                 rhs=wprod[:], start=True, stop=True)
                t1 = small.tile([128, D], f32, tag="t1")
                t2 = small.tile([128, D], f32, tag="t2")
                if with_bias:
                    b1 = small.tile([128, D], f32, tag="b1")
                    b2 = small.tile([128, D], f32, tag="b2")
                    nc.vector.tensor_tensor(out=b1[:nrow, :], in0=p1[:nrow, :],
                                            in1=bias[:nrow, :D],
                                            op=mybir.AluOpType.add)
                    nc.vector.tensor_tensor(out=b2[:nrow, :], in0=p2[:nrow, :],
                                            in1=bias[:nrow, D:],
                                            op=mybir.AluOpType.add)
                    nc.scalar.activation(t1[:nrow, :], b1[:nrow, :], **_act_kw())
                    nc.scalar.activation(t2[:nrow, :], b2[:nrow, :], **_act_kw())
                else:
                    nc.scalar.activation(t1[:nrow, :], p1[:nrow, :], **_act_kw())
                    # t2 = lrelu(p2) on DVE: max(x, slope*x)
                    ts2 = small.tile([128, D], f32, tag="ts2")
                    nc.vector.tensor_scalar_mul(ts2[:nrow, :], p2[:nrow, :],
                                                SLOPE if not SIM_RELU else 0.0)
                    nc.vector.tensor_tensor(out=t2[:nrow, :], in0=p2[:nrow, :],
                                            in1=ts2[:nrow, :],
                                            op=mybir.AluOpType.max)
                ob = small.tile([128, D], f32, tag="ob")
                nc.vector.tensor_tensor(out=ob[:nrow, :], in0=t1[:nrow, :],
                                        in1=t2[:nrow, :], op=mybir.AluOpType.add)
                nc.sync.dma_start(out=t_out[r0 : r0 + nrow, :], in_=ob[:nrow, :])

            prev_blocks = [0]
            for k in range(NCHUNK):
                w_lo, w_hi = k * CHUNK_W, min((k + 1) * CHUNK_W, NW)
                ntA = int(chunk_tile_lo[0, k + 1] - chunk_tile_lo[0, k])
                ntB = int(chunk_tile_lo[1, k + 1] - chunk_tile_lo[1, k])
                bufs = [None, None]
                sbufs = [None, None]
                # The 8 SWDGE sem lanes round-robin queue-unaware, so the
                # global queue sequence must be exactly periodic [0,1,2,3].
                # Each chunk issues 4 gather pieces; alternating the
                # interleave order by chunk parity balances queue load
                # (class A is ~2x class B).
                pieces = []  # (class, t0, t1)
                for x, nt in ((0, ntA), (1, ntB)):
                    assert nt >= 2, (k, x, nt)
                    h = nt // 2
                    pieces.append((x, 0, h))
                    pieces.append((x, h, nt))
                if k % 2 == 0:
                    order = [pieces[0], pieces[2], pieces[1], pieces[3]]
                else:
                    order = [pieces[2], pieces[0], pieces[3], pieces[1]]
                for x in range(2):
                    nt = (ntA, ntB)[x]
                    gpool, spool = ((gbufA, sSA), (gbufB, sSB))[x]
                    gb_t = gpool.tile([128, nt, D], bf, tag=f"g{x}", name=f"gbuf{x}")
                    bufs[x] = gb_t
                    col_base = (0 if x == 0 else TA) + int(chunk_tile_lo[x, k])
                    sb = spool.tile([128, nt, WSEG], bf, tag=f"s{x}", name=f"sbuf{x}")
                    nc.sync.dma_start(
                        out=sb[:],
                        in_=t_S[:, col_base * WSEG : (col_base + nt) * WSEG])
                    sbufs[x] = sb
                for qi, (x, t0, t1) in enumerate(order):
                    idxt = (idxA, idxB)[x]
                    srct = (srcA, srcB)[x]
                    s_lo = int(chunk_tile_lo[x, k] * 128)
                    n_idx = (t1 - t0) * 128
                    o = s_lo + t0 * 128
                    nc.gpsimd.dma_gather(
                        bufs[x][:, t0:t1, :], srct,
                        idxt[:, o // 16 : (o + n_idx) // 16],
                        n_idx, n_idx, D, single_packet=False,
                        queue_num=qi,
                    )

                for w in range(w_lo, w_hi):
                    psum = psw.tile([D, WSEG], f32, space="PSUM", tag="pw")
                    total_t = int(T[0][w] + T[1][w])
                    ti = 0
                    for x in range(2):
                        gtile0 = int(tile_base[x][w]) - int(chunk_tile_lo[x, k])
                        for t in range(int(T[x][w])):
                            nc.tensor.matmul(
                                psum[:],
                                lhsT=bufs[x][:, gtile0 + t, :],
                                rhs=sbufs[x][:, gtile0 + t, :],
                                start=(ti == 0),
                                stop=(ti == total_t - 1),
                            )
                            ti += 1
                    # X1T/X2T directly from psum (no separate drain)
                    o = w * WSEG
                    e = o + WSEG
                    nc.vector.tensor_tensor(
                        out=x2T[:, o:e], in0=embT[:, o:e], in1=psum[:],
                        op=mybir.AluOpType.mult)
                    nc.vector.tensor_tensor(
                        out=sideT[:, o:e], in0=embT[:, o:e], in1=psum[:],
                        op=mybir.AluOpType.add)

            for b in range(prev_blocks[0], NB):
                downstream_block(b)

    return nc


def kernel(embeddings, edge_row, edge_col, edge_val, W_sum, b_sum, W_prod,
           b_prod, _debug=False, _trace=False):
    embeddings = np.asarray(embeddings)
    edge_row = np.asarray(edge_row).astype(np.int64)
    edge_col = np.asarray(edge_col).astype(np.int64)
    edge_val = np.asarray(edge_val)
    W_sum = np.asarray(W_sum)
    W_prod = np.asarray(W_prod)
    b_sum = np.asarray(b_sum)
    b_prod = np.asarray(b_prod)

    per_core, sched = _preprocess(edge_row, edge_col, edge_val)
    with_bias = bool(np.any(b_sum) or np.any(b_prod))

    table_bf = embeddings.astype(BF16)
    wsumT = np.ascontiguousarray(W_sum.T).astype(BF16)
    wprodT = np.ascontiguousarray(W_prod.T).astype(BF16)
    if with_bias:
        biases = np.concatenate(
            [np.tile(b_sum[None, :], (128, 1)),
             np.tile(b_prod[None, :], (128, 1))], axis=1).astype(np.float32)

    nc = bacc.Bacc(num_swdge_queues=4)
    _build(nc, sched, with_bias)
    nc.compile()

    in_maps = []
    for c in range(NCORES):
        m = dict(
            table=np.asarray(table_bf),
            emb_own=np.asarray(table_bf[c * NPC : (c + 1) * NPC]),
            idxA=per_core[c]["idxA"],
            idxB=per_core[c]["idxB"],
            S=per_core[c]["S"],
            wsumT=np.asarray(wsumT),
            wprodT=np.asarray(wprodT),
        )
        if with_bias:
            m["biases"] = biases
        in_maps.append(m)

    res = run_bass_kernel_spmd(nc, in_maps, list(range(NCORES)),
                               trace=_trace)
    out = np.concatenate([res.results[c]["out"] for c in range(NCORES)], axis=0)
    if _debug:
        return out, res
    return out

